# revision 5
# baseline (speedup 1.0000x reference)
"""Groupwise projection kernel for Trainium2 (8 NeuronCores).

Problem: x [16, 4096, 512] fp32; 8 contiguous token segments per 4096-token
row, each with its own Linear (W [8, 512, 512], b [8, 512]);
out[b, t, :] = x[b, t, :] @ W[g(t)].T + b[g(t)].

Strategy:
  - Data-parallel over batch: 2 batch rows per core, weights replicated.
  - Host-side shard step lays x out transposed (d-major) so the contraction
    dim lands on SBUF partitions, and pre-rounds x/W to the fp32r format
    (fp32 with 11 mantissa bits, low 12 bits zero) so the TensorE runs its
    full-rate fp32r matmul path (1 cycle/row vs 4 for fp32).
  - Per core: out^T[o, t] = sum_k W^T[d_k, o]^T @ x^T[d_k, t] accumulated in
    PSUM over 4 k-blocks, bias added during the PSUM->SBUF copy.
  - Host gathers per-core out^T and transposes back.
"""

import sys

sys.path.insert(0, "/opt/trn_rl_repo")

import numpy as np
import concourse.bacc as bacc
import concourse.bass as bass
import concourse.mybir as mybir
import concourse.tile as tile
from concourse.bass_utils import run_bass_kernel_spmd

F32 = mybir.dt.float32
F32R = mybir.dt.float32r

LEN_GROUPS = (256, 512, 768, 384, 640, 512, 576, 448)
NUM_GROUPS, D_IN, D_OUT = 8, 512, 512
BATCH, T = 16, 4096
N_CORES = 8
ROWS_PER_CORE = BATCH // N_CORES  # 2
T_CORE = ROWS_PER_CORE * T  # 8192
KB = D_IN // 128  # 4 contraction blocks
OB = D_OUT // 128  # 4 output blocks

# Token tiles within one 4096-token row: (group, start, len), every len >= 256
# so the fp32r matmul runs at full rate (cost model: ap_size >= 256).
def _row_tiles():
    tiles = []
    start = 0
    for g, L in enumerate(LEN_GROUPS):
        if L <= 512:
            splits = [L]
        else:
            half = L // 2
            splits = [half, L - half]
        t = start
        for s in splits:
            tiles.append((g, t, s))
            t += s
        start += L
    return tiles


ROW_TILES = _row_tiles()

_NC_CACHE = None
_LAST_RESULTS = None  # test harness introspection (exec_time_ns etc.)


def _round_fp32r(a: np.ndarray) -> np.ndarray:
    """RNE-round fp32 to the fp32r format: 11 mantissa bits, low 12 bits 0."""
    u = np.ascontiguousarray(a).view(np.uint32)
    keep = u & np.uint32(0xFFFFF000)
    round_bit = (u >> np.uint32(12)) & np.uint32(1)
    lower = u & np.uint32(0xFFF)
    inc = (lower > 0x800) | ((lower == 0x800) & (round_bit == 1))
    out = keep + inc.astype(np.uint32) * np.uint32(0x1000)
    return out.view(np.float32)


def _build_nc():
    nc = bacc.Bacc("TRN2", target_bir_lowering=False, debug=False,
                   num_devices=N_CORES)

    xT = nc.dram_tensor("xT", [D_IN, T_CORE], F32R, kind="ExternalInput").ap()
    wT = nc.dram_tensor("wT", [NUM_GROUPS, D_IN, D_OUT], F32R,
                        kind="ExternalInput").ap()
    bT = nc.dram_tensor("bT", [128, NUM_GROUPS * OB], F32,
                        kind="ExternalInput").ap()
    outT = nc.dram_tensor("outT", [D_OUT, T_CORE], F32,
                          kind="ExternalOutput").ap()

    with tile.TileContext(nc) as tc:
        with (
            tc.tile_pool(name="wpool", bufs=1) as wpool,
            tc.tile_pool(name="bpool", bufs=1) as bpool,
            tc.tile_pool(name="xpool", bufs=4) as xpool,
            tc.tile_pool(name="opool", bufs=4) as opool,
            tc.tile_pool(name="psum", bufs=8, space=bass.MemorySpace.PSUM) as psum,
        ):
            # Weights resident in SBUF: [p, g, k, o] = wT[g][k*128+p, o].
            # Per-group DMAs are emitted just-in-time (before the first tile
            # that uses the group) so the first matmuls aren't stuck behind
            # the full 8.4MB weight load on the load ring.
            w_sb = wpool.tile([128, NUM_GROUPS, KB, D_OUT], F32R)
            b_sb = bpool.tile([128, NUM_GROUPS * OB], F32)
            nc.sync.dma_start(b_sb[:], bT)

            w_loaded = set()
            n_tile = 0
            for row in range(ROWS_PER_CORE):
                for g, t0, nt in ROW_TILES:
                    if g not in w_loaded:
                        w_loaded.add(g)
                        # weight loads ride the scalar HWDGE ring (free early)
                        nc.scalar.dma_start(
                            w_sb[:, g, :, :],
                            wT[g].rearrange("(k p) o -> p k o", p=128),
                        )
                    tt0 = row * T + t0
                    x_sb = xpool.tile([128, KB, 512], F32R, tag="x")
                    # x loads ride the sync HWDGE ring
                    nc.sync.dma_start(
                        x_sb[:, :, :nt],
                        xT[:, tt0:tt0 + nt].rearrange("(k p) t -> p k t", p=128),
                    )
                    o_sb = opool.tile([128, OB, 512], F32, tag="o")
                    for ob in range(OB):
                        acc = psum.tile([128, 512], F32, tag="acc")
                        for k in range(KB):
                            nc.tensor.matmul(
                                acc[:, :nt],
                                w_sb[:, g, k, ob * 128:(ob + 1) * 128],
                                x_sb[:, k, :nt],
                                start=(k == 0),
                                stop=(k == KB - 1),
                            )
                        # PSUM -> SBUF with bias, on DVE (ACT ring kept for DMA)
                        nc.vector.tensor_scalar_add(
                            o_sb[:, ob, :nt],
                            acc[:, :nt],
                            b_sb[:, g * OB + ob:g * OB + ob + 1],
                        )
                    # stores alternate between gpsimd SWDGE and scalar HWDGE
                    store_eng = nc.gpsimd if n_tile % 2 == 0 else nc.scalar
                    n_tile += 1
                    store_eng.dma_start(
                        outT[:, tt0:tt0 + nt].rearrange("(ob p) t -> p ob t", p=128),
                        o_sb[:, :, :nt],
                    )

    nc.compile()
    return nc


def kernel(x: np.ndarray, W: np.ndarray, b: np.ndarray) -> np.ndarray:
    global _NC_CACHE, _LAST_RESULTS
    x = np.asarray(x, dtype=np.float32)
    W = np.asarray(W, dtype=np.float32)
    b = np.asarray(b, dtype=np.float32)

    if _NC_CACHE is None:
        _NC_CACHE = _build_nc()
    nc = _NC_CACHE

    # W^T per group (rounded to fp32r), shared across cores.
    wT = _round_fp32r(np.ascontiguousarray(W.transpose(0, 2, 1)))
    # Bias laid out [p, g*4 + ob] = b[g, ob*128 + p].
    bT = np.ascontiguousarray(
        b.reshape(NUM_GROUPS, OB, 128).transpose(2, 0, 1).reshape(128, NUM_GROUPS * OB)
    )

    in_maps = []
    for c in range(N_CORES):
        xc = x[c * ROWS_PER_CORE:(c + 1) * ROWS_PER_CORE]  # [2, 4096, 512]
        xTc = _round_fp32r(
            np.ascontiguousarray(xc.reshape(T_CORE, D_IN).T)
        )  # [512, 8192]
        in_maps.append({"xT": xTc, "wT": wT, "bT": bT})

    res = run_bass_kernel_spmd(nc, in_maps, list(range(N_CORES)))
    _LAST_RESULTS = res

    out = np.empty((BATCH, T, D_OUT), dtype=np.float32)
    for c in range(N_CORES):
        oc = res.results[c]["outT"]  # [512, 8192]
        out[c * ROWS_PER_CORE:(c + 1) * ROWS_PER_CORE] = (
            oc.T.reshape(ROWS_PER_CORE, T, D_OUT)
        )
    return out


# revision 6
# speedup vs baseline: 1.0894x; 1.0894x over previous
"""Groupwise projection kernel for Trainium2 (8 NeuronCores).

Problem: x [16, 4096, 512] fp32; 8 contiguous token segments per 4096-token
row, each with its own Linear (W [8, 512, 512], b [8, 512]);
out[b, t, :] = x[b, t, :] @ W[g(t)].T + b[g(t)].

Strategy (v4):
  - The kernel is HBM-bound, so minimize per-core HBM bytes. Tokens are
    independent given their group, so the host reshuffles tokens freely.
    Each core processes 8192 tokens in 3 weight "slots" of (4096, 2560,
    1536) tokens; a slot uses one group's weight. The (core, slot) -> group
    assignment below tiles the global work exactly, so each core loads only
    3 of the 8 weight matrices (3.15MB instead of 8.4MB).
  - Host lays x out transposed (d-major) so the contraction dim lands on
    SBUF partitions, pre-rounded to the fp32r format (fp32 with 11 mantissa
    bits, low 12 bits zero) so TensorE runs the full-rate fp32r matmul path
    (1 cycle/row vs 4 for fp32).
  - Per core: out^T[o, 512t] = sum_k W^T[d_k, o]^T @ x^T[d_k, 512t]
    accumulated in PSUM over 4 k-blocks; bias added in the PSUM->SBUF copy
    on DVE. Loads ride the sync HWDGE ring; stores alternate between the
    gpsimd SWDGE and scalar HWDGE rings so loads/stores overlap.
  - Host scatters the per-core out^T back into the [16, 4096, 512] output.
"""

import sys

sys.path.insert(0, "/opt/trn_rl_repo")

import numpy as np
import concourse.bacc as bacc
import concourse.bass as bass
import concourse.mybir as mybir
import concourse.tile as tile
from concourse.bass_utils import run_bass_kernel_spmd

F32 = mybir.dt.float32
F32R = mybir.dt.float32r

LEN_GROUPS = (256, 512, 768, 384, 640, 512, 576, 448)
NUM_GROUPS, D_IN, D_OUT = 8, 512, 512
BATCH, T = 16, 4096
N_CORES = 8
T_CORE = 8192  # tokens per core (16*4096/8)
KB = D_IN // 128   # 4 contraction blocks
OB = D_OUT // 128  # 4 output blocks
NT = 512           # moving-dim tile (tokens per matmul)

# Weight slots per core: slot s holds SLOT_SIZES[s] tokens, all of one group.
SLOT_SIZES = (4096, 2560, 1536)
N_SLOTS = 3
# (slot, core) -> group. Tiles the 16*L_g tokens of every group exactly.
SLOT_GROUPS = (
    (0, 1, 1, 2, 2, 2, 6, 7),  # 4096-token slots
    (4, 4, 4, 4, 5, 5, 6, 6),  # 2560-token slots
    (3, 3, 3, 3, 5, 5, 7, 7),  # 1536-token slots
)

_NC_CACHE = None
_LAST_RESULTS = None  # test harness introspection (exec_time_ns etc.)


def _round_fp32r(a: np.ndarray) -> np.ndarray:
    """RNE-round fp32 to the fp32r format: 11 mantissa bits, low 12 bits 0."""
    u = np.ascontiguousarray(a).view(np.uint32)
    keep = u & np.uint32(0xFFFFF000)
    round_bit = (u >> np.uint32(12)) & np.uint32(1)
    lower = u & np.uint32(0xFFF)
    inc = (lower > 0x800) | ((lower == 0x800) & (round_bit == 1))
    out = keep + inc.astype(np.uint32) * np.uint32(0x1000)
    return out.view(np.float32)


def _token_assignment():
    """Per-core global token indices (into x.reshape(-1, 512)), slot-major."""
    starts = np.cumsum((0,) + LEN_GROUPS[:-1])
    pools = []
    for g in range(NUM_GROUPS):
        seg = np.arange(starts[g], starts[g] + LEN_GROUPS[g])
        pools.append(
            (np.arange(BATCH)[:, None] * T + seg[None, :]).reshape(-1)
        )
    used = [0] * NUM_GROUPS
    core_tok = [[] for _ in range(N_CORES)]
    for s in range(N_SLOTS):
        size = SLOT_SIZES[s]
        for c in range(N_CORES):
            g = SLOT_GROUPS[s][c]
            core_tok[c].append(pools[g][used[g]:used[g] + size])
            used[g] += size
    assert all(used[g] == BATCH * LEN_GROUPS[g] for g in range(NUM_GROUPS))
    return [np.concatenate(t) for t in core_tok]


TOKEN_INDEX = _token_assignment()


def _build_nc():
    nc = bacc.Bacc("TRN2", target_bir_lowering=False, debug=False,
                   num_devices=N_CORES)

    xT = nc.dram_tensor("xT", [D_IN, T_CORE], F32R, kind="ExternalInput").ap()
    wS = nc.dram_tensor("wS", [N_SLOTS, D_IN, D_OUT], F32R,
                        kind="ExternalInput").ap()
    bS = nc.dram_tensor("bS", [128, N_SLOTS * OB], F32,
                        kind="ExternalInput").ap()
    outT = nc.dram_tensor("outT", [D_OUT, T_CORE], F32,
                          kind="ExternalOutput").ap()

    n_chunks = T_CORE // 1024  # x staged in 2MB chunks of 1024 tokens

    with tile.TileContext(nc) as tc:
        with (
            tc.tile_pool(name="wpool", bufs=1) as wpool,
            tc.tile_pool(name="bpool", bufs=1) as bpool,
            tc.tile_pool(name="xpool", bufs=3) as xpool,
            tc.tile_pool(name="opool", bufs=3) as opool,
            tc.tile_pool(name="psum", bufs=8, space=bass.MemorySpace.PSUM) as psum,
        ):
            # Weights resident in SBUF: [p, s, k, o] = wS[s][k*128+p, o]
            w_sb = wpool.tile([128, N_SLOTS, KB, D_OUT], F32R)
            b_sb = bpool.tile([128, N_SLOTS * OB], F32)
            nc.sync.dma_start(b_sb[:], bS)

            w_loaded = set()
            x_chunks = [None] * n_chunks
            n_store = 0
            for i in range(T_CORE // NT):  # 16 tiles of 512 tokens
                # which slot does this tile belong to
                t0 = i * NT
                acc_t, s = 0, 0
                for s in range(N_SLOTS):
                    if t0 < acc_t + SLOT_SIZES[s]:
                        break
                    acc_t += SLOT_SIZES[s]
                if s not in w_loaded:
                    w_loaded.add(s)
                    # weight loads ride the scalar HWDGE ring (free early)
                    nc.scalar.dma_start(
                        w_sb[:, s, :, :],
                        wS[s].rearrange("(k p) o -> p k o", p=128),
                    )
                ci, co = t0 // 1024, t0 % 1024  # chunk idx / offset
                if x_chunks[ci] is None:
                    x_sb = xpool.tile([128, KB, 1024], F32R, tag="x")
                    # x loads ride the sync HWDGE ring
                    nc.sync.dma_start(
                        x_sb[:],
                        xT[:, ci * 1024:(ci + 1) * 1024]
                        .rearrange("(k p) t -> p k t", p=128),
                    )
                    x_chunks[ci] = x_sb
                x_sb = x_chunks[ci]
                o_sb = opool.tile([128, OB, NT], F32, tag="o")
                for ob in range(OB):
                    acc = psum.tile([128, NT], F32, tag="acc")
                    for k in range(KB):
                        nc.tensor.matmul(
                            acc[:],
                            w_sb[:, s, k, ob * 128:(ob + 1) * 128],
                            x_sb[:, k, co:co + NT],
                            start=(k == 0),
                            stop=(k == KB - 1),
                        )
                    # PSUM -> SBUF with bias, on DVE
                    nc.vector.tensor_scalar_add(
                        o_sb[:, ob, :],
                        acc[:],
                        b_sb[:, s * OB + ob:s * OB + ob + 1],
                    )
                # stores alternate between gpsimd SWDGE and scalar HWDGE
                store_eng = nc.gpsimd if n_store % 2 == 0 else nc.scalar
                n_store += 1
                store_eng.dma_start(
                    outT[:, t0:t0 + NT].rearrange("(ob p) t -> p ob t", p=128),
                    o_sb[:],
                )

    nc.compile()
    return nc


def kernel(x: np.ndarray, W: np.ndarray, b: np.ndarray) -> np.ndarray:
    global _NC_CACHE, _LAST_RESULTS
    x = np.asarray(x, dtype=np.float32)
    W = np.asarray(W, dtype=np.float32)
    b = np.asarray(b, dtype=np.float32)

    if _NC_CACHE is None:
        _NC_CACHE = _build_nc()
    nc = _NC_CACHE

    wT = _round_fp32r(np.ascontiguousarray(W.transpose(0, 2, 1)))  # [g, d, o]
    x_flat = x.reshape(BATCH * T, D_IN)

    in_maps = []
    for c in range(N_CORES):
        groups = [SLOT_GROUPS[s][c] for s in range(N_SLOTS)]
        wS = np.ascontiguousarray(wT[groups])  # [3, 512, 512]
        # bias laid out [p, s*4 + ob] = b[g_s, ob*128 + p]
        bS = np.ascontiguousarray(
            b[groups].reshape(N_SLOTS, OB, 128).transpose(2, 0, 1)
            .reshape(128, N_SLOTS * OB)
        )
        xTc = _round_fp32r(np.ascontiguousarray(x_flat[TOKEN_INDEX[c]].T))
        in_maps.append({"xT": xTc, "wS": wS, "bS": bS})

    res = run_bass_kernel_spmd(nc, in_maps, list(range(N_CORES)))
    _LAST_RESULTS = res

    out = np.empty((BATCH * T, D_OUT), dtype=np.float32)
    for c in range(N_CORES):
        out[TOKEN_INDEX[c]] = res.results[c]["outT"].T
    return out.reshape(BATCH, T, D_OUT)
